# revision 16
# baseline (speedup 1.0000x reference)
"""ABMIL attention pooling kernel for Trainium2 (8 NeuronCores, data-parallel over slides).

Per core: one slide x[N=16384, F=1024] f32.
  h = gelu(x @ W1 + b1)                    (N, 256)
  aV = tanh(h @ Wv + bv); aU = sigmoid(h @ Wu + bu)
  s = (aV*aU) @ Ww + bw; s += (mask-1)*BIG
  attn = softmax(s); out = attn @ h

Layout: transposed on chip (feature dims on partitions, tokens on the free axis)
so the matmul contractions sit on PE partitions. x tiles are transposed on the
PE (is_transpose matmuls vs identity, bf16).

Activation trick: sigmoid and exp are rewritten in terms of tanh so the whole
main loop uses one ACT function table set (gelu_and_others = {Gelu, Tanh, ...}):
  sigmoid(q) = (1 + tanh(q/2)) / 2        (the 1/2 is folded into Ww)
  exp(s)     = (1 + th) / (1 - th),  th = tanh(s/2)
Softmax weights are produced per chunk (flash-style, no max subtraction needed:
scores are O(1) bounded), so the weighted pooling overlaps the main loop.
"""

import os
import sys

for _p in ("/opt/trn_rl_repo", "/root/.axon_site/_ro/trn_rl_repo"):
    if os.path.isdir(_p) and _p not in sys.path:
        sys.path.insert(0, _p)

import numpy as np

import concourse.bass as bass
import concourse.bass_isa as bass_isa
import concourse.mybir as mybir
from concourse import bacc
from concourse import bass_utils as _bass_utils
from concourse.bass_utils import run_bass_kernel_spmd

# walrus's LDWEIGHTS optimization is disabled by default in this flow; the
# kernel is LDWEIGHTS-rate-bound, so turn it on for our compiles only.
_ENABLE_LDW_OPT = os.environ.get("KERNEL_LDW_OPT", "0") == "1"
if not getattr(_bass_utils, "_ldw_opt_patched", False):
    _orig_run_command = _bass_utils.run_command

    def _patched_run_command(argv, **kwargs):
        if _ENABLE_LDW_OPT and isinstance(argv, list):
            argv = [a.replace("--enable-ldw-opt=false", "--enable-ldw-opt=true")
                    if isinstance(a, str) else a for a in argv]
        return _orig_run_command(argv, **kwargs)

    _bass_utils.run_command = _patched_run_command
    _bass_utils._ldw_opt_patched = True
from concourse.masks import make_identity
from concourse.tile import TileContext

F32 = mybir.dt.float32
BF16 = mybir.dt.bfloat16
U8 = mybir.dt.uint8
AF = mybir.ActivationFunctionType
ALU = mybir.AluOpType

B = 8
N_TOK = 16384
F = 1024
H = 256
D = 128
KO = F // 128  # 8 feature blocks
HO = H // 2 // 128 * 2  # 2 hidden blocks
CH = 512       # tokens per chunk

_NC_CACHE = {}
LAST_RESULT = None


def build_nc(n_tok=N_TOK, use_gelu=True, dma_t=0):
    nch = n_tok // CH        # chunks
    J = n_tok // 128         # 128-token blocks
    HOx = 2
    nc = bacc.Bacc("TRN2", target_bir_lowering=False, debug=False)

    x_d = nc.dram_tensor("x", [n_tok, F], F32, kind="ExternalInput")
    mask_d = nc.dram_tensor("mask", [n_tok], U8, kind="ExternalInput")
    W1_d = nc.dram_tensor("W1", [F, H], F32, kind="ExternalInput")
    b1_d = nc.dram_tensor("b1", [H], F32, kind="ExternalInput")
    Wv_d = nc.dram_tensor("Wv", [H, D], F32, kind="ExternalInput")
    bv_d = nc.dram_tensor("bv", [D], F32, kind="ExternalInput")
    Wu_d = nc.dram_tensor("Wu", [H, D], F32, kind="ExternalInput")
    bu_d = nc.dram_tensor("bu", [D], F32, kind="ExternalInput")
    Ww_d = nc.dram_tensor("Ww", [D, 1], F32, kind="ExternalInput")
    bw_d = nc.dram_tensor("bw", [1], F32, kind="ExternalInput")
    tok_d = nc.dram_tensor("out_tok", [H], F32, kind="ExternalOutput")
    attn_d = nc.dram_tensor("out_attn", [n_tok], F32, kind="ExternalOutput")

    with TileContext(nc) as tc:
        with (
            tc.tile_pool(name="consts", bufs=1) as cpool,
            tc.tile_pool(name="xa", bufs=3) as xa_pool,
            tc.tile_pool(name="xt", bufs=2) as xt_pool,
            tc.tile_pool(name="act", bufs=3) as act_pool,
            tc.tile_pool(name="chain", bufs=5) as chain_pool,
            tc.tile_pool(name="psA", bufs=3, space="PSUM") as psA,   # transposes
            tc.tile_pool(name="psH", bufs=2, space="PSUM") as psH,   # hT accumulators
            tc.tile_pool(name="psV", bufs=2, space="PSUM") as psV,   # aV/aU
            tc.tile_pool(name="psS", bufs=1, space="PSUM") as psS,   # scores
        ):
            ident = cpool.tile([128, 128], BF16)
            make_identity(nc, ident)

            # ---------------- prefetch first x chunks before weights ----------------
            xa_head = []
            for c0 in range(min(3, nch)):
                xa0 = xa_pool.tile([128, 4, F], BF16, tag="xa", name=f"xa{c0}")
                nc.gpsimd.dma_start(
                    xa0, x_d[c0 * CH : (c0 + 1) * CH, :].rearrange("(s p) f -> p s f", p=128)
                )
                xa_head.append(xa0)

            # ---------------- constants / weights ----------------
            W1sb = cpool.tile([128, KO, H], BF16)
            nc.gpsimd.dma_start(W1sb, W1_d.ap().rearrange("(ko ki) h -> ki ko h", ki=128))
            Wvsb = cpool.tile([128, HOx, D], BF16)
            nc.gpsimd.dma_start(Wvsb, Wv_d.ap().rearrange("(ho hi) d -> hi ho d", hi=128))
            Wusb = cpool.tile([128, HOx, D], BF16)
            nc.gpsimd.dma_start(Wusb, Wu_d.ap().rearrange("(ho hi) d -> hi ho d", hi=128))
            Wwf = cpool.tile([128, 1], F32)
            nc.sync.dma_start(Wwf, Ww_d[:, :])
            Wwsb = cpool.tile([128, 1], BF16)      # Ww / 2 (sigmoid-as-tanh factor)
            nc.vector.tensor_scalar_mul(Wwsb, Wwf, 0.5)

            b1sb = cpool.tile([128, HOx], F32)
            for ho in range(HOx):
                nc.sync.dma_start(b1sb[:, ho : ho + 1],
                                  b1_d[ho * 128 : (ho + 1) * 128].rearrange("(a b) -> a b", b=1))
            bvsb = cpool.tile([128, 1], F32)
            nc.sync.dma_start(bvsb, bv_d.ap().rearrange("(a b) -> a b", b=1))
            busb = cpool.tile([128, 1], F32)
            nc.sync.dma_start(busb, bu_d.ap().rearrange("(a b) -> a b", b=1))
            buh = cpool.tile([128, 1], F32)        # bu / 2
            nc.vector.tensor_scalar_mul(buh, busb, 0.5)
            bwsb = cpool.tile([1, 1], F32)
            nc.sync.dma_start(bwsb, bw_d.ap().rearrange("(a b) -> a b", b=1))
            bw_b = cpool.tile([128, 1], F32)
            nc.gpsimd.partition_broadcast(bw_b, bwsb)
            bwh = cpool.tile([128, 1], F32)        # bw / 2
            nc.vector.tensor_scalar_mul(bwh, bw_b, 0.5)

            # mask -> additive term in score layout [q, j] (tok = j*128 + q)
            maskN = cpool.tile([J, 128], U8)
            nc.sync.dma_start(maskN, mask_d.ap().rearrange("(j q) -> j q", q=128))
            maskNbf = cpool.tile([J, 128], BF16)
            nc.vector.tensor_copy(maskNbf, maskN)
            maskT_ps = psA.tile([128, J], BF16, tag="tp")
            nc.tensor.transpose(maskT_ps, maskNbf, ident[:J, :J])
            maskterm = cpool.tile([128, J], F32)
            nc.scalar.activation(maskterm, maskT_ps, AF.Copy, bias=-100.0, scale=100.0)

            # ---------------- persistent state ----------------
            hT = cpool.tile([128, HOx, n_tok], BF16)      # gelu output, transposed
            Wflat = cpool.tile([1, n_tok], BF16)          # softmax weights, token order
            Zparts = cpool.tile([128, nch], F32)          # per-chunk sum(w) partials
            pool_parts = cpool.tile([128, HOx, nch], F32)
            pool_acc = cpool.tile([128, HOx], F32)
            s2s = {}   # staged chain tiles, keyed by chunk
            wcs = {}

            # ---------------- software-pipelined streaming loop ----------------
            # Stage A at chunk c; the softmax-weight chain is emitted with lags so
            # no in-order engine ever stalls on a cross-engine dependency:
            #   B1 (tanh-exp chain) at c-1, B2 (transpose+flatten) at c-2,
            #   B3 (broadcast + pooling) at c-3.
            def stageA(c):
                if c < len(xa_head):
                    xa = xa_head[c]
                else:
                    xa = xa_pool.tile([128, 4, F], BF16, tag="xa", name=f"xa{c}")
                    nc.gpsimd.dma_start(
                        xa, x_d[c * CH : (c + 1) * CH, :].rearrange("(s p) f -> p s f", p=128)
                    )
                xt = xt_pool.tile([128, KO, CH], BF16, tag="xt", name=f"xt{c}")
                ndma = 0
                for k in range(KO):
                    if (k % 2 == 1 and ndma < dma_t) or (KO - k <= dma_t - ndma):
                        # transpose via the DMA xbar (spare SDMA capacity)
                        ndma += 1
                        for s in range(4):
                            eng = nc.sync if s % 2 == 0 else nc.scalar
                            eng.dma_start(
                                xt[:, k, s * 128 : (s + 1) * 128],
                                xa[:, s, k * 128 : (k + 1) * 128],
                                transpose=True,
                            )
                        continue
                    tp = psA.tile([128, CH], BF16, tag="tp", name=f"tp{c}_{k}")
                    for s in range(4):
                        nc.tensor.transpose(
                            tp[:, s * 128 : (s + 1) * 128],
                            xa[:, s, k * 128 : (k + 1) * 128],
                            ident,
                        )
                    if k % 4 < 2:
                        nc.vector.tensor_copy(xt[:, k, :], tp)
                    else:
                        nc.scalar.copy(xt[:, k, :], tp)

                for half in range(HOx):
                    hps = psH.tile([128, CH], F32, tag="hps", name=f"hps{c}_{half}")
                    for k in range(KO):
                        nc.tensor.matmul(
                            hps,
                            W1sb[:, k, half * 128 : (half + 1) * 128],
                            xt[:, k, :],
                            start=(k == 0),
                            stop=(k == KO - 1),
                        )
                    nc.scalar.activation(
                        hT[:, half, c * CH : (c + 1) * CH], hps,
                        AF.Gelu if use_gelu else AF.Relu,
                        bias=b1sb[:, half : half + 1],
                    )

                # aV = tanh(. + bv);  t = tanh(./2 + bu/2)  [sigmoid via tanh]
                avps = psV.tile([128, CH], F32, tag="avps", name=f"av{c}")
                for half in range(HOx):
                    nc.tensor.matmul(
                        avps, Wvsb[:, half, :], hT[:, half, c * CH : (c + 1) * CH],
                        start=(half == 0), stop=(half == HOx - 1),
                    )
                av = act_pool.tile([128, CH], BF16, tag="av", name=f"avs{c}")
                nc.scalar.activation(av, avps, AF.Tanh, bias=bvsb)

                aups = psV.tile([128, CH], F32, tag="avps", name=f"au{c}")
                for half in range(HOx):
                    nc.tensor.matmul(
                        aups, Wusb[:, half, :], hT[:, half, c * CH : (c + 1) * CH],
                        start=(half == 0), stop=(half == HOx - 1),
                    )
                ut = act_pool.tile([128, CH], BF16, tag="ut", name=f"ut{c}")
                nc.scalar.activation(ut, aups, AF.Tanh, bias=buh, scale=0.5)

                # gated' = aV * (1 + t)   (the 1/2 lives in Wwsb)
                ut1 = act_pool.tile([128, CH], BF16, tag="ut1", name=f"ut1{c}")
                nc.vector.tensor_scalar_add(ut1, ut, 1.0)
                gt = act_pool.tile([128, CH], BF16, tag="gt", name=f"gt{c}")
                nc.vector.tensor_mul(gt, ut1, av)

                S_c = psS.tile([128, 4], F32, tag="sc", name=f"sc{c}")
                for s in range(4):
                    nc.tensor.matmul(
                        S_c[:, s : s + 1],
                        gt[:, s * 128 : (s + 1) * 128],
                        Wwsb,
                        start=True, stop=True,
                    )
                s2 = chain_pool.tile([128, 4], F32, tag="s2", name=f"s2_{c}")
                nc.vector.tensor_add(s2, S_c, maskterm[:, c * 4 : c * 4 + 4])
                s2s[c] = s2

            def stageB1(c):
                # w = exp(s + bw), via tanh: w = (1+th)/(1-th), th = tanh(s/2 + bw/2)
                s2 = s2s.pop(c)
                th = chain_pool.tile([128, 4], F32, tag="th", name=f"th{c}")
                nc.scalar.activation(th, s2, AF.Tanh, bias=bwh, scale=0.5)
                num = chain_pool.tile([128, 4], F32, tag="num", name=f"num{c}")
                nc.vector.tensor_scalar_add(num, th, 1.0)
                den = chain_pool.tile([128, 4], F32, tag="den", name=f"den{c}")
                nc.vector.tensor_scalar(den, th, -1.0, 1.0, ALU.mult, ALU.add)
                rec = chain_pool.tile([128, 4], F32, tag="rec", name=f"rec{c}")
                nc.vector.reciprocal(rec, den)
                wc = chain_pool.tile([128, 4], BF16, tag="wc", name=f"wc{c}")
                nc.vector.tensor_mul(wc, num, rec)
                nc.vector.tensor_reduce(Zparts[:, c : c + 1], wc,
                                        axis=mybir.AxisListType.X, op=ALU.add)
                wcs[c] = wc

            def stageB2(c):
                wc = wcs.pop(c)
                wt_ps = psA.tile([4, 128], BF16, tag="tp", name=f"wt{c}")
                nc.tensor.transpose(wt_ps, wc, ident)
                wstg = chain_pool.tile([4, 128], BF16, tag="wstg", name=f"wstg{c}")
                nc.vector.tensor_copy(wstg, wt_ps)
                nc.sync.dma_start(Wflat[:, c * CH : (c + 1) * CH], wstg)

            def stageB3(c):
                bcb = chain_pool.tile([128, CH], BF16, tag="bcb", name=f"bcb{c}")
                nc.gpsimd.partition_broadcast(bcb, Wflat[:, c * CH : (c + 1) * CH])
                scr0 = chain_pool.tile([128, CH], BF16, tag="scr0", name=f"scr0_{c}")
                nc.vector.scalar_tensor_tensor(
                    out=scr0, in0=hT[:, 0, c * CH : (c + 1) * CH], scalar=1.0,
                    in1=bcb, op0=ALU.mult, op1=ALU.mult,
                    accum_out=pool_parts[:, 0, c : c + 1],
                )
                scr1 = chain_pool.tile([128, CH], BF16, tag="scr1", name=f"scr1_{c}")
                nc.gpsimd.tensor_tensor(scr1, hT[:, 1, c * CH : (c + 1) * CH],
                                        bcb, ALU.mult)
                nc.vector.tensor_reduce(pool_parts[:, 1, c : c + 1], scr1,
                                        axis=mybir.AxisListType.X, op=ALU.add)

            for c in range(nch + 3):
                if c < nch:
                    stageA(c)
                if 0 <= c - 1 < nch:
                    stageB1(c - 1)
                if 0 <= c - 2 < nch:
                    stageB2(c - 2)
                if 0 <= c - 3 < nch:
                    stageB3(c - 3)

            # ---------------- finalize ----------------
            Zred = cpool.tile([128, 1], F32)
            nc.vector.tensor_reduce(Zred, Zparts, axis=mybir.AxisListType.X, op=ALU.add)
            Zall = cpool.tile([128, 1], F32)
            nc.gpsimd.partition_all_reduce(Zall, Zred, channels=128,
                                           reduce_op=bass_isa.ReduceOp.add)
            invZ = cpool.tile([128, 1], F32)
            nc.vector.reciprocal(invZ, Zall)

            WTall = cpool.tile([J, 128], BF16)
            nc.sync.dma_start(WTall, Wflat)
            attnT = cpool.tile([J, 128], F32)
            nc.scalar.activation(attnT, WTall, AF.Copy, bias=0.0, scale=invZ[:J, :])
            nc.sync.dma_start(attn_d.ap().rearrange("(j q) -> j q", q=128), attnT)

            pooled = cpool.tile([128, HOx], F32)
            for half in range(HOx):
                nc.vector.tensor_reduce(pool_acc[:, half : half + 1],
                                        pool_parts[:, half, :],
                                        axis=mybir.AxisListType.X, op=ALU.add)
            nc.scalar.activation(pooled, pool_acc, AF.Copy, bias=0.0, scale=invZ)
            for ho in range(HOx):
                nc.sync.dma_start(
                    tok_d[ho * 128 : (ho + 1) * 128].rearrange("(a b) -> a b", b=1),
                    pooled[:, ho : ho + 1],
                )

    nc.compile()
    return nc


def _get_nc(n_tok=N_TOK):
    if n_tok not in _NC_CACHE:
        _NC_CACHE[n_tok] = build_nc(n_tok)
    return _NC_CACHE[n_tok]


def kernel(x, mask, W1, b1, Wv, bv, Wu, bu, Ww, bw):
    global LAST_RESULT
    x = np.ascontiguousarray(np.asarray(x, dtype=np.float32))
    mask_u8 = np.ascontiguousarray(np.asarray(mask).astype(np.uint8))
    W1 = np.ascontiguousarray(np.asarray(W1, dtype=np.float32))
    b1 = np.ascontiguousarray(np.asarray(b1, dtype=np.float32))
    Wv = np.ascontiguousarray(np.asarray(Wv, dtype=np.float32))
    bv = np.ascontiguousarray(np.asarray(bv, dtype=np.float32))
    Wu = np.ascontiguousarray(np.asarray(Wu, dtype=np.float32))
    bu = np.ascontiguousarray(np.asarray(bu, dtype=np.float32))
    Ww = np.ascontiguousarray(np.asarray(Ww, dtype=np.float32))
    bw = np.ascontiguousarray(np.asarray(bw, dtype=np.float32))

    n_tok = x.shape[1]
    nc = _get_nc(n_tok)
    in_maps = []
    for b in range(B):
        in_maps.append({
            "x": x[b], "mask": mask_u8[b],
            "W1": W1, "b1": b1, "Wv": Wv, "bv": bv,
            "Wu": Wu, "bu": bu, "Ww": Ww, "bw": bw,
        })
    res = run_bass_kernel_spmd(nc, in_maps, core_ids=list(range(B)))
    LAST_RESULT = res
    toks = np.stack([r["out_tok"] for r in res.results])[:, None, :]
    attn = np.stack([r["out_attn"] for r in res.results])[:, None, :]
    return toks.astype(np.float32), attn.astype(np.float32)


# revision 17
# speedup vs baseline: 2.1433x; 2.1433x over previous
"""ABMIL attention pooling kernel for Trainium2 (8 NeuronCores, data-parallel over slides).

Per core: one slide x[N=16384, F=1024] f32.
  h = gelu(x @ W1 + b1)                    (N, 256)
  aV = tanh(h @ Wv + bv); aU = sigmoid(h @ Wu + bu)
  s = (aV*aU) @ Ww + bw; s += (mask-1)*BIG
  attn = softmax(s); out = attn @ h

Layout: transposed on chip (feature dims on partitions, tokens on the free axis)
so the matmul contractions sit on PE partitions. x tiles are transposed on the
PE (is_transpose matmuls vs identity, bf16).

Activation trick: sigmoid and exp are rewritten in terms of tanh so the whole
main loop uses one ACT function table set (gelu_and_others = {Gelu, Tanh, ...}):
  sigmoid(q) = (1 + tanh(q/2)) / 2        (the 1/2 is folded into Ww)
  exp(s)     = (1 + th) / (1 - th),  th = tanh(s/2)
Softmax weights are produced per chunk (flash-style, no max subtraction needed:
scores are O(1) bounded), so the weighted pooling overlaps the main loop.
"""

import os
import sys

for _p in ("/opt/trn_rl_repo", "/root/.axon_site/_ro/trn_rl_repo"):
    if os.path.isdir(_p) and _p not in sys.path:
        sys.path.insert(0, _p)

import numpy as np

import concourse.bass as bass
import concourse.bass_isa as bass_isa
import concourse.mybir as mybir
from concourse import bacc
from concourse import bass_utils as _bass_utils
from concourse.bass_utils import run_bass_kernel_spmd

# walrus's LDWEIGHTS optimization is disabled by default in this flow; the
# kernel is LDWEIGHTS-rate-bound, so turn it on for our compiles only.
_ENABLE_LDW_OPT = os.environ.get("KERNEL_LDW_OPT", "0") == "1"
if not getattr(_bass_utils, "_ldw_opt_patched", False):
    _orig_run_command = _bass_utils.run_command

    def _patched_run_command(argv, **kwargs):
        if _ENABLE_LDW_OPT and isinstance(argv, list):
            argv = [a.replace("--enable-ldw-opt=false", "--enable-ldw-opt=true")
                    if isinstance(a, str) else a for a in argv]
        return _orig_run_command(argv, **kwargs)

    _bass_utils.run_command = _patched_run_command
    _bass_utils._ldw_opt_patched = True
from concourse.masks import make_identity
from concourse.tile import TileContext

F32 = mybir.dt.float32
BF16 = mybir.dt.bfloat16
U8 = mybir.dt.uint8
AF = mybir.ActivationFunctionType
ALU = mybir.AluOpType

B = 8
N_TOK = 16384
F = 1024
H = 256
D = 128
KO = F // 128  # 8 feature blocks
HO = H // 2 // 128 * 2  # 2 hidden blocks
CH = 512       # tokens per chunk

_NC_CACHE = {}
LAST_RESULT = None


def build_nc(n_tok=N_TOK, use_gelu=True, dma_t=0):
    nch = n_tok // CH        # chunks
    J = n_tok // 128         # 128-token blocks
    HOx = 2
    nc = bacc.Bacc("TRN2", target_bir_lowering=False, debug=False)

    x_d = nc.dram_tensor("x", [n_tok, F], F32, kind="ExternalInput")
    mask_d = nc.dram_tensor("mask", [n_tok], U8, kind="ExternalInput")
    W1_d = nc.dram_tensor("W1", [F, H], F32, kind="ExternalInput")
    b1_d = nc.dram_tensor("b1", [H], F32, kind="ExternalInput")
    Wv_d = nc.dram_tensor("Wv", [H, D], F32, kind="ExternalInput")
    bv_d = nc.dram_tensor("bv", [D], F32, kind="ExternalInput")
    Wu_d = nc.dram_tensor("Wu", [H, D], F32, kind="ExternalInput")
    bu_d = nc.dram_tensor("bu", [D], F32, kind="ExternalInput")
    Ww_d = nc.dram_tensor("Ww", [D, 1], F32, kind="ExternalInput")
    bw_d = nc.dram_tensor("bw", [1], F32, kind="ExternalInput")
    tok_d = nc.dram_tensor("out_tok", [H], F32, kind="ExternalOutput")
    attn_d = nc.dram_tensor("out_attn", [n_tok], F32, kind="ExternalOutput")

    with TileContext(nc) as tc:
        with (
            tc.tile_pool(name="consts", bufs=1) as cpool,
            tc.tile_pool(name="xa", bufs=3) as xa_pool,
            tc.tile_pool(name="xt", bufs=2) as xt_pool,
            tc.tile_pool(name="act", bufs=3) as act_pool,
            tc.tile_pool(name="chain", bufs=5) as chain_pool,
            tc.tile_pool(name="psA", bufs=3, space="PSUM") as psA,   # transposes
            tc.tile_pool(name="psH", bufs=2, space="PSUM") as psH,   # hT accumulators
            tc.tile_pool(name="psV", bufs=2, space="PSUM") as psV,   # aV/aU
            tc.tile_pool(name="psS", bufs=1, space="PSUM") as psS,   # scores
        ):
            ident = cpool.tile([128, 128], BF16)
            make_identity(nc, ident)

            # ---------------- prefetch first x chunks before weights ----------------
            xa_head = []
            for c0 in range(min(3, nch)):
                xa0 = xa_pool.tile([128, 4, F], BF16, tag="xa", name=f"xa{c0}")
                nc.gpsimd.dma_start(
                    xa0, x_d[c0 * CH : (c0 + 1) * CH, :].rearrange("(s p) f -> p s f", p=128)
                )
                xa_head.append(xa0)

            # ---------------- constants / weights ----------------
            W1sb = cpool.tile([128, KO, H], BF16)
            nc.gpsimd.dma_start(W1sb, W1_d.ap().rearrange("(ko ki) h -> ki ko h", ki=128))
            Wvsb = cpool.tile([128, HOx, D], BF16)
            nc.gpsimd.dma_start(Wvsb, Wv_d.ap().rearrange("(ho hi) d -> hi ho d", hi=128))
            Wusb = cpool.tile([128, HOx, D], BF16)
            nc.gpsimd.dma_start(Wusb, Wu_d.ap().rearrange("(ho hi) d -> hi ho d", hi=128))
            Wwf = cpool.tile([128, 1], F32)
            nc.sync.dma_start(Wwf, Ww_d[:, :])
            Wwsb = cpool.tile([128, 1], BF16)      # Ww / 2 (sigmoid-as-tanh factor)
            nc.vector.tensor_scalar_mul(Wwsb, Wwf, 0.5)

            b1sb = cpool.tile([128, HOx], F32)
            for ho in range(HOx):
                nc.sync.dma_start(b1sb[:, ho : ho + 1],
                                  b1_d[ho * 128 : (ho + 1) * 128].rearrange("(a b) -> a b", b=1))
            bvsb = cpool.tile([128, 1], F32)
            nc.sync.dma_start(bvsb, bv_d.ap().rearrange("(a b) -> a b", b=1))
            busb = cpool.tile([128, 1], F32)
            nc.sync.dma_start(busb, bu_d.ap().rearrange("(a b) -> a b", b=1))
            buh = cpool.tile([128, 1], F32)        # bu / 2
            nc.vector.tensor_scalar_mul(buh, busb, 0.5)
            bwsb = cpool.tile([1, 1], F32)
            nc.sync.dma_start(bwsb, bw_d.ap().rearrange("(a b) -> a b", b=1))
            bw_b = cpool.tile([128, 1], F32)
            nc.gpsimd.partition_broadcast(bw_b, bwsb)
            bwh = cpool.tile([128, 1], F32)        # bw / 2
            nc.vector.tensor_scalar_mul(bwh, bw_b, 0.5)

            # mask -> additive term in score layout [q, j] (tok = j*128 + q)
            maskN = cpool.tile([J, 128], U8)
            nc.sync.dma_start(maskN, mask_d.ap().rearrange("(j q) -> j q", q=128))
            maskNbf = cpool.tile([J, 128], BF16)
            nc.vector.tensor_copy(maskNbf, maskN)
            maskT_ps = psA.tile([128, J], BF16, tag="tp")
            nc.tensor.transpose(maskT_ps, maskNbf, ident[:J, :J])
            maskterm = cpool.tile([128, J], F32)
            nc.scalar.activation(maskterm, maskT_ps, AF.Copy, bias=-100.0, scale=100.0)

            # ---------------- persistent state ----------------
            hT = cpool.tile([128, HOx, n_tok], BF16)      # gelu output, transposed
            Wflat = cpool.tile([1, n_tok], BF16)          # softmax weights, token order
            Zparts = cpool.tile([128, nch], F32)          # per-chunk sum(w) partials
            pool_parts = cpool.tile([128, HOx, nch], F32)
            pool_acc = cpool.tile([128, HOx], F32)
            s2s = {}   # staged chain tiles, keyed by chunk
            wcs = {}

            # ---------------- software-pipelined streaming loop ----------------
            # Stage A at chunk c; the softmax-weight chain is emitted with lags so
            # no in-order engine ever stalls on a cross-engine dependency:
            #   B1 (tanh-exp chain) at c-1, B2 (transpose+flatten) at c-2,
            #   B3 (broadcast + pooling) at c-3.
            def stageA(c):
                if c < len(xa_head):
                    xa = xa_head[c]
                else:
                    xa = xa_pool.tile([128, 4, F], BF16, tag="xa", name=f"xa{c}")
                    nc.gpsimd.dma_start(
                        xa, x_d[c * CH : (c + 1) * CH, :].rearrange("(s p) f -> p s f", p=128)
                    )
                xt = xt_pool.tile([128, KO, CH], BF16, tag="xt", name=f"xt{c}")
                ndma = 0
                for k in range(KO):
                    if (k % 2 == 1 and ndma < dma_t) or (KO - k <= dma_t - ndma):
                        # transpose via the DMA xbar (spare SDMA capacity)
                        ndma += 1
                        for s in range(4):
                            eng = nc.sync if s % 2 == 0 else nc.scalar
                            eng.dma_start(
                                xt[:, k, s * 128 : (s + 1) * 128],
                                xa[:, s, k * 128 : (k + 1) * 128],
                                transpose=True,
                            )
                        continue
                    tp = psA.tile([128, CH], BF16, tag="tp", name=f"tp{c}_{k}")
                    for s in range(4):
                        nc.tensor.transpose(
                            tp[:, s * 128 : (s + 1) * 128],
                            xa[:, s, k * 128 : (k + 1) * 128],
                            ident,
                        )
                    if k % 4 < 2:
                        nc.vector.tensor_copy(xt[:, k, :], tp)
                    else:
                        nc.scalar.copy(xt[:, k, :], tp)

                for half in range(HOx):
                    hps = psH.tile([128, CH], F32, tag="hps", name=f"hps{c}_{half}")
                    for k in range(KO):
                        nc.tensor.matmul(
                            hps,
                            W1sb[:, k, half * 128 : (half + 1) * 128],
                            xt[:, k, :],
                            start=(k == 0),
                            stop=(k == KO - 1),
                        )
                    nc.scalar.activation(
                        hT[:, half, c * CH : (c + 1) * CH], hps,
                        AF.Gelu if use_gelu else AF.Relu,
                        bias=b1sb[:, half : half + 1],
                    )

                # aV = tanh(. + bv);  t = tanh(./2 + bu/2)  [sigmoid via tanh]
                avps = psV.tile([128, CH], F32, tag="avps", name=f"av{c}")
                for half in range(HOx):
                    nc.tensor.matmul(
                        avps, Wvsb[:, half, :], hT[:, half, c * CH : (c + 1) * CH],
                        start=(half == 0), stop=(half == HOx - 1),
                    )
                av = act_pool.tile([128, CH], BF16, tag="av", name=f"avs{c}")
                nc.scalar.activation(av, avps, AF.Tanh, bias=bvsb)

                aups = psV.tile([128, CH], F32, tag="avps", name=f"au{c}")
                for half in range(HOx):
                    nc.tensor.matmul(
                        aups, Wusb[:, half, :], hT[:, half, c * CH : (c + 1) * CH],
                        start=(half == 0), stop=(half == HOx - 1),
                    )
                ut = act_pool.tile([128, CH], BF16, tag="ut", name=f"ut{c}")
                nc.scalar.activation(ut, aups, AF.Tanh, bias=buh, scale=0.5)

                # gated' = aV * (1 + t)   (the 1/2 lives in Wwsb)
                ut1 = act_pool.tile([128, CH], BF16, tag="ut1", name=f"ut1{c}")
                nc.vector.tensor_scalar_add(ut1, ut, 1.0)
                gt = act_pool.tile([128, CH], BF16, tag="gt", name=f"gt{c}")
                nc.vector.tensor_mul(gt, ut1, av)

                S_c = psS.tile([128, 4], F32, tag="sc", name=f"sc{c}")
                for s in range(4):
                    nc.tensor.matmul(
                        S_c[:, s : s + 1],
                        gt[:, s * 128 : (s + 1) * 128],
                        Wwsb,
                        start=True, stop=True,
                    )
                s2 = chain_pool.tile([128, 4], F32, tag="s2", name=f"s2_{c}")
                nc.vector.tensor_add(s2, S_c, maskterm[:, c * 4 : c * 4 + 4])
                s2s[c] = s2

            def stageB1(c):
                # w = exp(s + bw), via tanh: w = (1+th)/(1-th), th = tanh(s/2 + bw/2)
                s2 = s2s.pop(c)
                th = chain_pool.tile([128, 4], F32, tag="th", name=f"th{c}")
                nc.scalar.activation(th, s2, AF.Tanh, bias=bwh, scale=0.5)
                num = chain_pool.tile([128, 4], F32, tag="num", name=f"num{c}")
                nc.vector.tensor_scalar_add(num, th, 1.0)
                den = chain_pool.tile([128, 4], F32, tag="den", name=f"den{c}")
                nc.vector.tensor_scalar(den, th, -1.0, 1.0, ALU.mult, ALU.add)
                rec = chain_pool.tile([128, 4], F32, tag="rec", name=f"rec{c}")
                nc.vector.reciprocal(rec, den)
                wc = chain_pool.tile([128, 4], BF16, tag="wc", name=f"wc{c}")
                nc.vector.tensor_mul(wc, num, rec)
                nc.vector.tensor_reduce(Zparts[:, c : c + 1], wc,
                                        axis=mybir.AxisListType.X, op=ALU.add)
                wcs[c] = wc

            def stageB2(c):
                wc = wcs.pop(c)
                wt_ps = psA.tile([4, 128], BF16, tag="tp", name=f"wt{c}")
                nc.tensor.transpose(wt_ps, wc, ident)
                wstg = chain_pool.tile([4, 128], BF16, tag="wstg", name=f"wstg{c}")
                nc.vector.tensor_copy(wstg, wt_ps)
                nc.sync.dma_start(Wflat[:, c * CH : (c + 1) * CH], wstg)

            def stageB3(c):
                bcb = chain_pool.tile([128, CH], BF16, tag="bcb", name=f"bcb{c}")
                nc.gpsimd.partition_broadcast(bcb, Wflat[:, c * CH : (c + 1) * CH])
                scr0 = chain_pool.tile([128, CH], BF16, tag="scr0", name=f"scr0_{c}")
                nc.vector.scalar_tensor_tensor(
                    out=scr0, in0=hT[:, 0, c * CH : (c + 1) * CH], scalar=1.0,
                    in1=bcb, op0=ALU.mult, op1=ALU.mult,
                    accum_out=pool_parts[:, 0, c : c + 1],
                )
                scr1 = chain_pool.tile([128, CH], BF16, tag="scr1", name=f"scr1_{c}")
                nc.vector.scalar_tensor_tensor(
                    out=scr1, in0=hT[:, 1, c * CH : (c + 1) * CH], scalar=1.0,
                    in1=bcb, op0=ALU.mult, op1=ALU.mult,
                    accum_out=pool_parts[:, 1, c : c + 1],
                )

            for c in range(nch + 3):
                if c < nch:
                    stageA(c)
                if 0 <= c - 1 < nch:
                    stageB1(c - 1)
                if 0 <= c - 2 < nch:
                    stageB2(c - 2)
                if 0 <= c - 3 < nch:
                    stageB3(c - 3)

            # ---------------- finalize ----------------
            Zred = cpool.tile([128, 1], F32)
            nc.vector.tensor_reduce(Zred, Zparts, axis=mybir.AxisListType.X, op=ALU.add)
            Zall = cpool.tile([128, 1], F32)
            nc.gpsimd.partition_all_reduce(Zall, Zred, channels=128,
                                           reduce_op=bass_isa.ReduceOp.add)
            invZ = cpool.tile([128, 1], F32)
            nc.vector.reciprocal(invZ, Zall)

            WTall = cpool.tile([J, 128], BF16)
            nc.sync.dma_start(WTall, Wflat)
            attnT = cpool.tile([J, 128], F32)
            nc.scalar.activation(attnT, WTall, AF.Copy, bias=0.0, scale=invZ[:J, :])
            nc.sync.dma_start(attn_d.ap().rearrange("(j q) -> j q", q=128), attnT)

            pooled = cpool.tile([128, HOx], F32)
            for half in range(HOx):
                nc.vector.tensor_reduce(pool_acc[:, half : half + 1],
                                        pool_parts[:, half, :],
                                        axis=mybir.AxisListType.X, op=ALU.add)
            nc.scalar.activation(pooled, pool_acc, AF.Copy, bias=0.0, scale=invZ)
            for ho in range(HOx):
                nc.sync.dma_start(
                    tok_d[ho * 128 : (ho + 1) * 128].rearrange("(a b) -> a b", b=1),
                    pooled[:, ho : ho + 1],
                )

    nc.compile()
    return nc


def _get_nc(n_tok=N_TOK):
    if n_tok not in _NC_CACHE:
        _NC_CACHE[n_tok] = build_nc(n_tok)
    return _NC_CACHE[n_tok]


def kernel(x, mask, W1, b1, Wv, bv, Wu, bu, Ww, bw):
    global LAST_RESULT
    x = np.ascontiguousarray(np.asarray(x, dtype=np.float32))
    mask_u8 = np.ascontiguousarray(np.asarray(mask).astype(np.uint8))
    W1 = np.ascontiguousarray(np.asarray(W1, dtype=np.float32))
    b1 = np.ascontiguousarray(np.asarray(b1, dtype=np.float32))
    Wv = np.ascontiguousarray(np.asarray(Wv, dtype=np.float32))
    bv = np.ascontiguousarray(np.asarray(bv, dtype=np.float32))
    Wu = np.ascontiguousarray(np.asarray(Wu, dtype=np.float32))
    bu = np.ascontiguousarray(np.asarray(bu, dtype=np.float32))
    Ww = np.ascontiguousarray(np.asarray(Ww, dtype=np.float32))
    bw = np.ascontiguousarray(np.asarray(bw, dtype=np.float32))

    n_tok = x.shape[1]
    nc = _get_nc(n_tok)
    in_maps = []
    for b in range(B):
        in_maps.append({
            "x": x[b], "mask": mask_u8[b],
            "W1": W1, "b1": b1, "Wv": Wv, "bv": bv,
            "Wu": Wu, "bu": bu, "Ww": Ww, "bw": bw,
        })
    res = run_bass_kernel_spmd(nc, in_maps, core_ids=list(range(B)))
    LAST_RESULT = res
    toks = np.stack([r["out_tok"] for r in res.results])[:, None, :]
    attn = np.stack([r["out_attn"] for r in res.results])[:, None, :]
    return toks.astype(np.float32), attn.astype(np.float32)


# revision 19
# speedup vs baseline: 2.4715x; 1.1532x over previous
"""ABMIL attention pooling kernel for Trainium2 (8 NeuronCores, data-parallel over slides).

Per core: one slide x[N=16384, F=1024] f32.
  h = gelu(x @ W1 + b1)                    (N, 256)
  aV = tanh(h @ Wv + bv); aU = sigmoid(h @ Wu + bu)
  s = (aV*aU) @ Ww + bw; s += (mask-1)*BIG
  attn = softmax(s); out = attn @ h

Layout: transposed on chip (feature dims on partitions, tokens on the free axis)
so the matmul contractions sit on PE partitions. x tiles are transposed on the
PE (is_transpose matmuls vs identity, bf16).

Activation trick: sigmoid and exp are rewritten in terms of tanh so the whole
main loop uses one ACT function table set (gelu_and_others = {Gelu, Tanh, ...}):
  sigmoid(q) = (1 + tanh(q/2)) / 2        (the 1/2 is folded into Ww)
  exp(s)     = (1 + th) / (1 - th),  th = tanh(s/2)
Softmax weights are produced per chunk (flash-style, no max subtraction needed:
scores are O(1) bounded), so the weighted pooling overlaps the main loop.
"""

import os
import sys

for _p in ("/opt/trn_rl_repo", "/root/.axon_site/_ro/trn_rl_repo"):
    if os.path.isdir(_p) and _p not in sys.path:
        sys.path.insert(0, _p)

import numpy as np

import concourse.bass as bass
import concourse.bass_isa as bass_isa
import concourse.mybir as mybir
from concourse import bacc
from concourse import bass_utils as _bass_utils
from concourse.bass_utils import run_bass_kernel_spmd

# walrus's LDWEIGHTS optimization is disabled by default in this flow; the
# kernel is LDWEIGHTS-rate-bound, so turn it on for our compiles only.
_ENABLE_LDW_OPT = os.environ.get("KERNEL_LDW_OPT", "0") == "1"
if not getattr(_bass_utils, "_ldw_opt_patched", False):
    _orig_run_command = _bass_utils.run_command

    def _patched_run_command(argv, **kwargs):
        if _ENABLE_LDW_OPT and isinstance(argv, list):
            argv = [a.replace("--enable-ldw-opt=false", "--enable-ldw-opt=true")
                    if isinstance(a, str) else a for a in argv]
        return _orig_run_command(argv, **kwargs)

    _bass_utils.run_command = _patched_run_command
    _bass_utils._ldw_opt_patched = True
from concourse.masks import make_identity
from concourse.tile import TileContext

F32 = mybir.dt.float32
BF16 = mybir.dt.bfloat16
FP8 = mybir.dt.float8e4
U8 = mybir.dt.uint8
AF = mybir.ActivationFunctionType
ALU = mybir.AluOpType

B = 8
N_TOK = 16384
F = 1024
H = 256
D = 128
KO = F // 128  # 8 feature blocks
HO = H // 2 // 128 * 2  # 2 hidden blocks
CH = 512       # tokens per chunk

_NC_CACHE = {}
LAST_RESULT = None


def build_nc(n_tok=N_TOK, use_gelu=True, fp8=True):
    nch = n_tok // CH        # chunks
    J = n_tok // 128         # 128-token blocks
    HOx = 2
    nc = bacc.Bacc("TRN2", target_bir_lowering=False, debug=False)

    x_d = nc.dram_tensor("x", [n_tok, F], F32, kind="ExternalInput")
    mask_d = nc.dram_tensor("mask", [n_tok], U8, kind="ExternalInput")
    W1_d = nc.dram_tensor("W1", [F, H], F32, kind="ExternalInput")
    b1_d = nc.dram_tensor("b1", [H], F32, kind="ExternalInput")
    Wv_d = nc.dram_tensor("Wv", [H, D], F32, kind="ExternalInput")
    bv_d = nc.dram_tensor("bv", [D], F32, kind="ExternalInput")
    Wu_d = nc.dram_tensor("Wu", [H, D], F32, kind="ExternalInput")
    bu_d = nc.dram_tensor("bu", [D], F32, kind="ExternalInput")
    Ww_d = nc.dram_tensor("Ww", [D, 1], F32, kind="ExternalInput")
    bw_d = nc.dram_tensor("bw", [1], F32, kind="ExternalInput")
    tok_d = nc.dram_tensor("out_tok", [H], F32, kind="ExternalOutput")
    attn_d = nc.dram_tensor("out_attn", [n_tok], F32, kind="ExternalOutput")

    with TileContext(nc) as tc:
        with (
            tc.tile_pool(name="consts", bufs=1) as cpool,
            tc.tile_pool(name="xa", bufs=3) as xa_pool,
            tc.tile_pool(name="xt", bufs=2) as xt_pool,
            tc.tile_pool(name="act", bufs=3) as act_pool,
            tc.tile_pool(name="chain", bufs=5) as chain_pool,
            tc.tile_pool(name="psA", bufs=3, space="PSUM") as psA,   # transposes
            tc.tile_pool(name="psH", bufs=2, space="PSUM") as psH,   # hT accumulators
            tc.tile_pool(name="psV", bufs=2, space="PSUM") as psV,   # aV/aU
            tc.tile_pool(name="psS", bufs=1, space="PSUM") as psS,   # scores
        ):
            # ---------------- constants / weights ----------------
            W1bf = cpool.tile([128, KO, H], BF16)
            nc.gpsimd.dma_start(W1bf, W1_d.ap().rearrange("(ko ki) h -> ki ko h", ki=128))
            if fp8:
                W1sb = cpool.tile([128, KO, H], FP8)
                nc.vector.tensor_scalar_mul(W1sb, W1bf, 16.0)
            else:
                W1sb = W1bf
            Wvsb = cpool.tile([128, HOx, D], BF16)
            nc.gpsimd.dma_start(Wvsb, Wv_d.ap().rearrange("(ho hi) d -> hi ho d", hi=128))
            Wusb = cpool.tile([128, HOx, D], BF16)
            nc.gpsimd.dma_start(Wusb, Wu_d.ap().rearrange("(ho hi) d -> hi ho d", hi=128))
            Wwf = cpool.tile([128, 1], F32)
            nc.sync.dma_start(Wwf, Ww_d[:, :])
            Wwsb = cpool.tile([128, 1], BF16)      # Ww / 2 (sigmoid-as-tanh factor)
            nc.vector.tensor_scalar_mul(Wwsb, Wwf, 0.5)

            b1sb = cpool.tile([128, HOx], F32)
            for ho in range(HOx):
                nc.sync.dma_start(b1sb[:, ho : ho + 1],
                                  b1_d[ho * 128 : (ho + 1) * 128].rearrange("(a b) -> a b", b=1))
            bvsb = cpool.tile([128, 1], F32)
            nc.sync.dma_start(bvsb, bv_d.ap().rearrange("(a b) -> a b", b=1))
            busb = cpool.tile([128, 1], F32)
            nc.sync.dma_start(busb, bu_d.ap().rearrange("(a b) -> a b", b=1))
            buh = cpool.tile([128, 1], F32)        # bu / 2
            nc.vector.tensor_scalar_mul(buh, busb, 0.5)
            bwsb = cpool.tile([1, 1], F32)
            nc.sync.dma_start(bwsb, bw_d.ap().rearrange("(a b) -> a b", b=1))
            bw_b = cpool.tile([128, 1], F32)
            nc.gpsimd.partition_broadcast(bw_b, bwsb)
            bwh = cpool.tile([128, 1], F32)        # bw / 2
            nc.vector.tensor_scalar_mul(bwh, bw_b, 0.5)

            ident = cpool.tile([128, 128], BF16)
            make_identity(nc, ident)

            # mask -> additive term in score layout [q, j] (tok = j*128 + q)
            maskN = cpool.tile([J, 128], U8)
            nc.sync.dma_start(maskN, mask_d.ap().rearrange("(j q) -> j q", q=128))
            maskNbf = cpool.tile([J, 128], BF16)
            nc.vector.tensor_copy(maskNbf, maskN)
            maskT_ps = psA.tile([128, J], BF16, tag="tp")
            nc.tensor.transpose(maskT_ps, maskNbf, ident[:J, :J])
            maskterm = cpool.tile([128, J], F32)
            nc.scalar.activation(maskterm, maskT_ps, AF.Copy, bias=-100.0, scale=100.0)

            # ---------------- persistent state ----------------
            hT = cpool.tile([128, HOx, n_tok], BF16)      # gelu output, transposed
            Wflat = cpool.tile([1, n_tok], BF16)          # softmax weights, token order
            Zparts = cpool.tile([128, nch], F32)          # per-chunk sum(w) partials
            pool_parts = cpool.tile([128, HOx, nch], F32)
            pool_acc = cpool.tile([128, HOx], F32)
            s2s = {}   # staged chain tiles, keyed by chunk
            wcs = {}

            # ---------------- software-pipelined streaming loop ----------------
            # Stage A at chunk c; the softmax-weight chain is emitted with lags so
            # no in-order engine ever stalls on a cross-engine dependency:
            #   B1 (tanh-exp chain) at c-1, B2 (transpose+flatten) at c-2,
            #   B3 (broadcast + pooling) at c-3.
            def stageA(c):
                xa = xa_pool.tile([128, 4, F], BF16, tag="xa", name=f"xa{c}")
                nc.gpsimd.dma_start(
                    xa, x_d[c * CH : (c + 1) * CH, :].rearrange("(s p) f -> p s f", p=128)
                )
                xt = xt_pool.tile([128, KO, CH], FP8 if fp8 else BF16, tag="xt", name=f"xt{c}")
                for k in range(KO):
                    tp = psA.tile([128, CH], BF16, tag="tp", name=f"tp{c}_{k}")
                    for s in range(4):
                        nc.tensor.transpose(
                            tp[:, s * 128 : (s + 1) * 128],
                            xa[:, s, k * 128 : (k + 1) * 128],
                            ident,
                        )
                    if k % 4 < 2:
                        nc.vector.tensor_copy(xt[:, k, :], tp)
                    else:
                        nc.scalar.copy(xt[:, k, :], tp)

                for half in range(HOx):
                    hps = psH.tile([128, CH], F32, tag="hps", name=f"hps{c}_{half}")
                    if fp8:
                        for kk in range(KO // 2):
                            nc.tensor.matmul(
                                hps,
                                W1sb[:, 2 * kk : 2 * kk + 2, half * 128 : (half + 1) * 128],
                                xt[:, 2 * kk : 2 * kk + 2, :],
                                start=(kk == 0),
                                stop=(kk == KO // 2 - 1),
                                perf_mode=mybir.MatmulPerfMode.DoubleRow,
                            )
                    else:
                        for k in range(KO):
                            nc.tensor.matmul(
                                hps,
                                W1sb[:, k, half * 128 : (half + 1) * 128],
                                xt[:, k, :],
                                start=(k == 0),
                                stop=(k == KO - 1),
                            )
                    nc.scalar.activation(
                        hT[:, half, c * CH : (c + 1) * CH], hps,
                        AF.Gelu if use_gelu else AF.Relu,
                        bias=b1sb[:, half : half + 1],
                        scale=(1.0 / 16.0) if fp8 else 1.0,
                    )

                # aV = tanh(. + bv);  t = tanh(./2 + bu/2)  [sigmoid via tanh]
                avps = psV.tile([128, CH], F32, tag="avps", name=f"av{c}")
                for half in range(HOx):
                    nc.tensor.matmul(
                        avps, Wvsb[:, half, :], hT[:, half, c * CH : (c + 1) * CH],
                        start=(half == 0), stop=(half == HOx - 1),
                    )
                av = act_pool.tile([128, CH], BF16, tag="av", name=f"avs{c}")
                nc.scalar.activation(av, avps, AF.Tanh, bias=bvsb)

                aups = psV.tile([128, CH], F32, tag="avps", name=f"au{c}")
                for half in range(HOx):
                    nc.tensor.matmul(
                        aups, Wusb[:, half, :], hT[:, half, c * CH : (c + 1) * CH],
                        start=(half == 0), stop=(half == HOx - 1),
                    )
                ut = act_pool.tile([128, CH], BF16, tag="ut", name=f"ut{c}")
                nc.scalar.activation(ut, aups, AF.Tanh, bias=buh, scale=0.5)

                # gated' = aV * (1 + t)   (the 1/2 lives in Wwsb)
                ut1 = act_pool.tile([128, CH], BF16, tag="ut1", name=f"ut1{c}")
                nc.vector.tensor_scalar_add(ut1, ut, 1.0)
                gt = act_pool.tile([128, CH], BF16, tag="gt", name=f"gt{c}")
                nc.vector.tensor_mul(gt, ut1, av)

                S_c = psS.tile([128, 4], F32, tag="sc", name=f"sc{c}")
                for s in range(4):
                    nc.tensor.matmul(
                        S_c[:, s : s + 1],
                        gt[:, s * 128 : (s + 1) * 128],
                        Wwsb,
                        start=True, stop=True,
                    )
                s2 = chain_pool.tile([128, 4], F32, tag="s2", name=f"s2_{c}")
                nc.vector.tensor_add(s2, S_c, maskterm[:, c * 4 : c * 4 + 4])
                s2s[c] = s2

            def stageB1(c):
                # w = exp(s + bw), via tanh: w = (1+th)/(1-th), th = tanh(s/2 + bw/2)
                s2 = s2s.pop(c)
                th = chain_pool.tile([128, 4], F32, tag="th", name=f"th{c}")
                nc.scalar.activation(th, s2, AF.Tanh, bias=bwh, scale=0.5)
                num = chain_pool.tile([128, 4], F32, tag="num", name=f"num{c}")
                nc.vector.tensor_scalar_add(num, th, 1.0)
                den = chain_pool.tile([128, 4], F32, tag="den", name=f"den{c}")
                nc.vector.tensor_scalar(den, th, -1.0, 1.0, ALU.mult, ALU.add)
                rec = chain_pool.tile([128, 4], F32, tag="rec", name=f"rec{c}")
                nc.vector.reciprocal(rec, den)
                wc = chain_pool.tile([128, 4], BF16, tag="wc", name=f"wc{c}")
                nc.vector.tensor_mul(wc, num, rec)
                nc.vector.tensor_reduce(Zparts[:, c : c + 1], wc,
                                        axis=mybir.AxisListType.X, op=ALU.add)
                wcs[c] = wc

            def stageB2(c):
                wc = wcs.pop(c)
                wt_ps = psA.tile([4, 128], BF16, tag="tp", name=f"wt{c}")
                nc.tensor.transpose(wt_ps, wc, ident)
                wstg = chain_pool.tile([4, 128], BF16, tag="wstg", name=f"wstg{c}")
                nc.vector.tensor_copy(wstg, wt_ps)
                nc.sync.dma_start(Wflat[:, c * CH : (c + 1) * CH], wstg)

            def stageB3(c):
                bcb = chain_pool.tile([128, CH], BF16, tag="bcb", name=f"bcb{c}")
                nc.gpsimd.partition_broadcast(bcb, Wflat[:, c * CH : (c + 1) * CH])
                scr0 = chain_pool.tile([128, CH], BF16, tag="scr0", name=f"scr0_{c}")
                nc.vector.scalar_tensor_tensor(
                    out=scr0, in0=hT[:, 0, c * CH : (c + 1) * CH], scalar=1.0,
                    in1=bcb, op0=ALU.mult, op1=ALU.mult,
                    accum_out=pool_parts[:, 0, c : c + 1],
                )
                scr1 = chain_pool.tile([128, CH], BF16, tag="scr1", name=f"scr1_{c}")
                nc.vector.scalar_tensor_tensor(
                    out=scr1, in0=hT[:, 1, c * CH : (c + 1) * CH], scalar=1.0,
                    in1=bcb, op0=ALU.mult, op1=ALU.mult,
                    accum_out=pool_parts[:, 1, c : c + 1],
                )

            for c in range(nch + 3):
                if c < nch:
                    stageA(c)
                if 0 <= c - 1 < nch:
                    stageB1(c - 1)
                if 0 <= c - 2 < nch:
                    stageB2(c - 2)
                if 0 <= c - 3 < nch:
                    stageB3(c - 3)

            # ---------------- finalize ----------------
            Zred = cpool.tile([128, 1], F32)
            nc.vector.tensor_reduce(Zred, Zparts, axis=mybir.AxisListType.X, op=ALU.add)
            Zall = cpool.tile([128, 1], F32)
            nc.gpsimd.partition_all_reduce(Zall, Zred, channels=128,
                                           reduce_op=bass_isa.ReduceOp.add)
            invZ = cpool.tile([128, 1], F32)
            nc.vector.reciprocal(invZ, Zall)

            WTall = cpool.tile([J, 128], BF16)
            nc.sync.dma_start(WTall, Wflat)
            attnT = cpool.tile([J, 128], F32)
            nc.scalar.activation(attnT, WTall, AF.Copy, bias=0.0, scale=invZ[:J, :])
            nc.sync.dma_start(attn_d.ap().rearrange("(j q) -> j q", q=128), attnT)

            pooled = cpool.tile([128, HOx], F32)
            for half in range(HOx):
                nc.vector.tensor_reduce(pool_acc[:, half : half + 1],
                                        pool_parts[:, half, :],
                                        axis=mybir.AxisListType.X, op=ALU.add)
            nc.scalar.activation(pooled, pool_acc, AF.Copy, bias=0.0, scale=invZ)
            for ho in range(HOx):
                nc.sync.dma_start(
                    tok_d[ho * 128 : (ho + 1) * 128].rearrange("(a b) -> a b", b=1),
                    pooled[:, ho : ho + 1],
                )

    nc.compile()
    return nc


def _get_nc(n_tok=N_TOK):
    if n_tok not in _NC_CACHE:
        _NC_CACHE[n_tok] = build_nc(n_tok)
    return _NC_CACHE[n_tok]


def kernel(x, mask, W1, b1, Wv, bv, Wu, bu, Ww, bw):
    global LAST_RESULT
    x = np.ascontiguousarray(np.asarray(x, dtype=np.float32))
    mask_u8 = np.ascontiguousarray(np.asarray(mask).astype(np.uint8))
    W1 = np.ascontiguousarray(np.asarray(W1, dtype=np.float32))
    b1 = np.ascontiguousarray(np.asarray(b1, dtype=np.float32))
    Wv = np.ascontiguousarray(np.asarray(Wv, dtype=np.float32))
    bv = np.ascontiguousarray(np.asarray(bv, dtype=np.float32))
    Wu = np.ascontiguousarray(np.asarray(Wu, dtype=np.float32))
    bu = np.ascontiguousarray(np.asarray(bu, dtype=np.float32))
    Ww = np.ascontiguousarray(np.asarray(Ww, dtype=np.float32))
    bw = np.ascontiguousarray(np.asarray(bw, dtype=np.float32))

    n_tok = x.shape[1]
    nc = _get_nc(n_tok)
    in_maps = []
    for b in range(B):
        in_maps.append({
            "x": x[b], "mask": mask_u8[b],
            "W1": W1, "b1": b1, "Wv": Wv, "bv": bv,
            "Wu": Wu, "bu": bu, "Ww": Ww, "bw": bw,
        })
    res = run_bass_kernel_spmd(nc, in_maps, core_ids=list(range(B)))
    LAST_RESULT = res
    toks = np.stack([r["out_tok"] for r in res.results])[:, None, :]
    attn = np.stack([r["out_attn"] for r in res.results])[:, None, :]
    return toks.astype(np.float32), attn.astype(np.float32)
